# revision 1
# baseline (speedup 1.0000x reference)
"""Trainium2 Bass kernel for nn_PairwisePredictionHead.

Math (reference):
  xd = x @ W_down.T + b_down             # [L, 128]
  q, k = xd[:, :64], xd[:, 64:]
  h[i,j,:] = W1p @ (q_j*k_i) + W1d @ (q_j - k_i) + b1    # [L, L, 128]
  g = gelu_exact(h)
  out = W2 @ LN(g) + b2                   # [L, L, 64]

Sharding: row-shard i across 8 cores (96 rows each). Each core gets the full
q-side (all 768 j) plus its own 96 k-rows; cores are independent (no
collectives), outputs concatenated on host.

Per-core device algorithm (layout: h on partitions, pairs on free):
  - lhsT_i = [[W1p.T * k_i[:,None]] ; W1d.T]  (only top half rebuilt per i)
  - psum1[h, j] = lhsT_i.T @ [q.T; q.T]            (PE, N=768)
  - g = Gelu(psum1 + (b1 - W1d@k_i))               (ACT, bf16 out)
  - g2 = g*g                                       (DVE / ACT alternating)
  - per 128-j chunk c: po[j, 66c:66c+65] = (g_c as stationary) @ [W2z.T|1]
                       po[j, 66c+65]    = (g2_c as stationary) @ [1]
    W2z = (W2*ln_g) - rowmean: zero-mean rows absorb LN's mean subtraction
    into the weights (w.(g-mu) == (w-mean(w)).g).
  - stats: mu = Sg/128, m2 = Sg2/128, var = m2 - mu^2
           r = (var+eps)^-1/2, s = (var+eps)^+1/2  (gpsimd pow)
  - fixup: po[j, nb-cols] += s[j]*c[nb]  (rank-1 matmul; c = W2@ln_b + b2;
           after the final r-scale this contributes exactly +c)
  - out[j, nb] = r[j] * po[j, nb-cols]   (DVE, one pass per i)
"""

import os
from contextlib import ExitStack

import numpy as np
import ml_dtypes

import concourse.bass as bass
import concourse.mybir as mybir
import concourse.tile as tile
from concourse import bacc
from concourse.bass_utils import run_bass_kernel_spmd
from concourse.masks import make_identity

F32 = mybir.dt.float32
BF16 = mybir.dt.bfloat16
ALU = mybir.AluOpType
AF = mybir.ActivationFunctionType

B, L, D = 1, 768, 1024
DP, H, NB = 128, 128, 64
NCORES = 8
ROWS = L // NCORES  # 96 pair-grid rows per core
P = 128
EPS = 1e-5
SB = 2  # i's per stats batch (bounded by PSUM banks)


def _build(nc):
    xT = nc.dram_tensor("xT", [D, L], F32, kind="ExternalInput")
    xTr = nc.dram_tensor("xTr", [D, ROWS], F32, kind="ExternalInput")
    WdTq = nc.dram_tensor("WdTq", [D, 64], F32, kind="ExternalInput")
    WdTk = nc.dram_tensor("WdTk", [D, 64], F32, kind="ExternalInput")
    bdq = nc.dram_tensor("bdq", [64, 1], F32, kind="ExternalInput")
    bdk = nc.dram_tensor("bdk", [64, 1], F32, kind="ExternalInput")
    W1pT = nc.dram_tensor("W1pT", [64, P], F32, kind="ExternalInput")
    W1dT = nc.dram_tensor("W1dT", [64, P], F32, kind="ExternalInput")
    b1v = nc.dram_tensor("b1v", [P, 1], F32, kind="ExternalInput")
    W2zTe = nc.dram_tensor("W2zTe", [P, 65], BF16, kind="ExternalInput")
    cblk = nc.dram_tensor("cblk", [6, 384], BF16, kind="ExternalInput")
    out = nc.dram_tensor("out", [ROWS, L, NB], F32, kind="ExternalOutput")

    with tile.TileContext(nc) as tc, ExitStack() as ctx:
        const = ctx.enter_context(tc.tile_pool(name="const", bufs=1))
        work = ctx.enter_context(tc.tile_pool(name="work", bufs=4))
        outp = ctx.enter_context(tc.tile_pool(name="outp", bufs=4))
        statsp = ctx.enter_context(tc.tile_pool(name="statsp", bufs=3))
        pp1 = ctx.enter_context(tc.tile_pool(name="pp1", bufs=2, space="PSUM"))
        ppo = ctx.enter_context(tc.tile_pool(name="ppo", bufs=3, space="PSUM"))
        ppt = ctx.enter_context(tc.tile_pool(name="ppt", bufs=1, space="PSUM"))

        # ---- constants into SBUF ----
        xT_sb = const.tile([P, 8, L], F32)
        nc.sync.dma_start(out=xT_sb, in_=xT[:].rearrange("(c p) l -> p c l", p=P))
        xTr_sb = const.tile([P, 8, ROWS], F32)
        nc.sync.dma_start(out=xTr_sb, in_=xTr[:].rearrange("(c p) r -> p c r", p=P))
        WdTq_sb = const.tile([P, 8, 64], F32)
        nc.sync.dma_start(out=WdTq_sb, in_=WdTq[:].rearrange("(c p) m -> p c m", p=P))
        WdTk_sb = const.tile([P, 8, 64], F32)
        nc.sync.dma_start(out=WdTk_sb, in_=WdTk[:].rearrange("(c p) m -> p c m", p=P))
        bdq_sb = const.tile([64, 1], F32)
        nc.sync.dma_start(out=bdq_sb, in_=bdq[:])
        bdk_sb = const.tile([64, 1], F32)
        nc.sync.dma_start(out=bdk_sb, in_=bdk[:])
        W1pT_sb = const.tile([64, P], F32)
        nc.sync.dma_start(out=W1pT_sb, in_=W1pT[:])
        W1dT_sb = const.tile([64, P], F32)
        nc.sync.dma_start(out=W1dT_sb, in_=W1dT[:])
        b1v_sb = const.tile([P, 1], F32)
        nc.sync.dma_start(out=b1v_sb, in_=b1v[:])
        W2zTe_sb = const.tile([P, 65], BF16)
        nc.sync.dma_start(out=W2zTe_sb, in_=W2zTe[:])
        cblk_sb = const.tile([6, 384], BF16)
        nc.sync.dma_start(out=cblk_sb, in_=cblk[:])
        identity = const.tile([P, P], F32)
        make_identity(nc, identity)
        mhalf = const.tile([P, SB * 6], F32)
        nc.vector.memset(mhalf, -0.5)
        phalf = const.tile([P, SB * 6], F32)
        nc.vector.memset(phalf, 0.5)

        # ---- prep: qq = [q.T; q.T], kT (local rows), b1c = b1 - W1d@kT ----
        qq = const.tile([P, L], F32)
        kT_sb = const.tile([64, ROWS], F32)
        b1c = const.tile([P, ROWS], F32)

        pq = pp1.tile([64, L], F32, tag="p1")
        for c in range(8):
            for h0, h1 in ((0, 512), (512, 768)):
                nc.tensor.matmul(
                    pq[:, h0:h1], WdTq_sb[:, c, :], xT_sb[:, c, h0:h1],
                    start=(c == 0), stop=(c == 7),
                )
        nc.scalar.activation(qq[0:64, :], pq, AF.Identity, bias=bdq_sb)
        nc.sync.dma_start(out=qq[64:128, :], in_=qq[0:64, :])

        pk = ppo.tile([64, ROWS], F32, tag="po")
        for c in range(8):
            nc.tensor.matmul(pk, WdTk_sb[:, c, :], xTr_sb[:, c, :],
                             start=(c == 0), stop=(c == 7))
        nc.scalar.activation(kT_sb, pk, AF.Identity, bias=bdk_sb)

        pc = ppo.tile([P, ROWS], F32, tag="po")
        nc.tensor.matmul(pc, W1dT_sb, kT_sb, start=True, stop=True)
        nc.scalar.activation(b1c, pc, AF.Identity, bias=b1v_sb, scale=-1.0)

        # persistent W1 stationary tiles (bottom halves static = W1d.T)
        lhsT_t = [const.tile([P, P], F32, tag=f"lhsT{t}", name=f"lhsT{t}")
                  for t in range(2)]
        for t in range(2):
            nc.sync.dma_start(out=lhsT_t[t][64:128, :], in_=W1dT[:])

        # ---- main loop (per-i stats: short dep chains, deep pipelining) ----
        for ii in range(ROWS):
            lt = lhsT_t[ii % 2]
            nc.vector.tensor_scalar_mul(lt[0:64, :], W1pT_sb, kT_sb[:, ii:ii + 1])

            p1 = pp1.tile([P, L], F32, tag="p1", name="p1")
            nc.tensor.matmul(p1[:, 0:512], lt, qq[:, 0:512], start=True, stop=True)
            nc.tensor.matmul(p1[:, 512:768], lt, qq[:, 512:768],
                             start=True, stop=True)

            g = work.tile([P, L], BF16, tag="g", name="g")
            nc.scalar.activation(g, p1, AF.Gelu, bias=b1c[:, ii:ii + 1])
            g2 = work.tile([P, L], BF16, tag="g2", name="g2")
            if ii % 2 == 0:
                nc.vector.tensor_mul(g2, g, g)
            else:
                nc.scalar.square(g2, g)

            po = ppo.tile([P, 396], F32, tag="po", name="po")
            for c in range(6):
                nc.tensor.matmul(po[:, c * 66:c * 66 + 65],
                                 g[:, c * 128:(c + 1) * 128], W2zTe_sb,
                                 start=(c == 0), stop=False)
                nc.tensor.matmul(po[:, c * 66 + 65:c * 66 + 66],
                                 g2[:, c * 128:(c + 1) * 128],
                                 W2zTe_sb[:, 64:65],
                                 start=False, stop=False)

            # stats: [mu | m2] = [Sg | Sg2]/128
            stage = statsp.tile([P, 6, 2], F32, tag="stage", name="stage")
            po_stats = po[:].rearrange("p (c w) -> p c w", w=66)[:, :, 64:66]
            nc.vector.tensor_scalar_mul(stage, po_stats, 1.0 / 128.0)
            muv = stage[:, :, 0]
            m2v = stage[:, :, 1]
            mu2 = statsp.tile([P, 6], F32, tag="mu2", name="mu2")
            nc.vector.tensor_tensor(mu2, muv, muv, ALU.mult)
            veps = statsp.tile([P, 6], F32, tag="veps", name="veps")
            nc.vector.scalar_tensor_tensor(veps, m2v, EPS, mu2[:],
                                           ALU.add, ALU.subtract)
            r_sb = statsp.tile([P, 6], F32, tag="r", name="r")
            s_sb = statsp.tile([P, 6], F32, tag="s", name="s")
            nc.gpsimd.tensor_tensor(r_sb, veps[:], mhalf[:, 0:6], ALU.pow)
            nc.gpsimd.tensor_tensor(s_sb, veps[:], phalf[:, 0:6], ALU.pow)

            pt = ppt.tile([6, P], F32, tag="pt", name="pt")
            nc.tensor.transpose(pt, s_sb[:], identity)
            sT_bf = statsp.tile([6, P], BF16, tag="sT", name="sT")
            nc.vector.tensor_copy(sT_bf, pt)

            po_main = po[:].rearrange("p (c w) -> p c w", w=66)[:, :, 0:64]
            nc.tensor.matmul(po_main, sT_bf[:], cblk_sb, start=False, stop=True)
            o_sb = outp.tile([P, 6, 64], F32, tag="osb", name="osb")
            rb = r_sb[:, :, None].broadcast_to([P, 6, 64])
            nc.vector.tensor_mul(o_sb, po_main, rb)
            nc.sync.dma_start(
                out=out[ii].rearrange("(c p) n -> p c n", p=P), in_=o_sb)


def host_prep(x, W_down, b_down, W1, b1, ln_g, ln_b, W2, b2):
    f32 = np.float32
    bf16 = ml_dtypes.bfloat16
    xTfull = np.ascontiguousarray(x[0].T.astype(f32))  # [D, L]
    common = {
        "xT": xTfull,
        "WdTq": np.ascontiguousarray(W_down[:64, :].T.astype(f32)),
        "WdTk": np.ascontiguousarray(W_down[64:, :].T.astype(f32)),
        "bdq": np.ascontiguousarray(b_down[:64].astype(f32).reshape(64, 1)),
        "bdk": np.ascontiguousarray(b_down[64:].astype(f32).reshape(64, 1)),
        "W1pT": np.ascontiguousarray(W1[:, :64].T.astype(f32)),
        "W1dT": np.ascontiguousarray(W1[:, 64:].T.astype(f32)),
        "b1v": np.ascontiguousarray(b1.astype(f32).reshape(P, 1)),
    }
    W2g = W2.astype(np.float64) * ln_g.astype(np.float64)[None, :]
    W2z = W2g - W2g.mean(axis=1, keepdims=True)
    W2zTe = np.concatenate([W2z.T, np.ones((P, 1))], axis=1)  # [128, 65]
    common["W2zTe"] = np.ascontiguousarray(W2zTe.astype(bf16))
    cvec = W2.astype(np.float64) @ ln_b.astype(np.float64) + b2.astype(np.float64)
    cb = np.zeros((6, 384), dtype=np.float64)
    for c in range(6):
        cb[c, c * 64:(c + 1) * 64] = cvec
    common["cblk"] = np.ascontiguousarray(cb.astype(bf16))
    return common, xTfull


def kernel(x, W_down, b_down, W1, b1, ln_g, ln_b, W2, b2):
    x = np.asarray(x)
    common, xTfull = host_prep(
        x, np.asarray(W_down), np.asarray(b_down), np.asarray(W1),
        np.asarray(b1), np.asarray(ln_g), np.asarray(ln_b), np.asarray(W2),
        np.asarray(b2))

    nc = bacc.Bacc("TRN2")
    _build(nc)
    nc.finalize()

    in_maps = []
    for core in range(NCORES):
        m = dict(common)
        i0 = core * ROWS
        m["xTr"] = np.ascontiguousarray(xTfull[:, i0:i0 + ROWS])
        in_maps.append(m)

    trace = os.environ.get("KERNEL_TRACE", "0") == "1"
    res = run_bass_kernel_spmd(nc, in_maps, core_ids=list(range(NCORES)),
                               trace=trace)
    if trace and res.exec_time_ns is not None:
        print(f"HW exec time: {res.exec_time_ns} ns")
    outs = [res.results[c]["out"] for c in range(NCORES)]
    full = np.concatenate(outs, axis=0)  # [768, 768, 64]
    return full[None].astype(np.float32)



# revision 11
# speedup vs baseline: 2.4012x; 2.4012x over previous
"""Trainium2 Bass kernel for nn_PairwisePredictionHead.

Math (reference):
  xd = x @ W_down.T + b_down             # [L, 128]
  q, k = xd[:, :64], xd[:, 64:]
  h[i,j,:] = W1p @ (q_j*k_i) + W1d @ (q_j - k_i) + b1    # [L, L, 128]
  g = gelu_exact(h)
  out = W2 @ LN(g) + b2                   # [L, L, 64]

Sharding: row-shard i across 8 cores (96 rows each). Each core gets the full
q-side (all 768 j) plus its own 96 k-rows; cores are independent (no
collectives), outputs concatenated on host.

Device (per core, per i; all matmuls bf16, 512/256-col splits for PSUM banks):
  - lhsT_i = [[W1pT * k_i[:,None]] ; W1dT]  (top half rebuilt per i on DVE)
  - p1[h, j]   = lhsT_i.T @ [q.T; q.T]            (PE, N=768)
  - g  = Gelu(p1 + (b1 - W1d@k_i))                (ACT, bf16 out)
  - g2 = g*g                                      (DVE, bf16)
  - pA[0:65, j]  = [W2z.T*ln_g | ones].T @ g      (PE; row 64 = sum_h g)
  - pA[96, j]    = ones.T @ g2                    (PE; sum_h g^2)
  - DMA pA[0:65] -> dev_out[i], pA[96:97] -> dev_s2[i]   (fp32, 3KB descs)

Host tail (vectorized numpy): mu = Sg/128, var = Sg2/128 - mu^2,
  r = rsqrt(var+eps), out[i,j,:] = dev_out[i,:,j]*r + (W2@ln_b + b2).
W2z rows are zero-meaned so the matmul absorbs LN's mean subtraction
(w.(g-mu) == (w-mean(w)).g).
"""

import os
from contextlib import ExitStack

import numpy as np
import ml_dtypes

import concourse.bass as bass
import concourse.mybir as mybir
import concourse.tile as tile
from concourse import bacc
from concourse.bass_utils import run_bass_kernel_spmd

F32 = mybir.dt.float32
BF16 = mybir.dt.bfloat16
ALU = mybir.AluOpType
AF = mybir.ActivationFunctionType

B, L, D = 1, 768, 1024
DP, H, NB = 128, 128, 64
NCORES = 8
ROWS = L // NCORES  # 96 pair-grid rows per core
P = 128
EPS = 1e-5


def _build(nc):
    xT = nc.dram_tensor("xT", [D, L], BF16, kind="ExternalInput")
    xTr = nc.dram_tensor("xTr", [D, ROWS], BF16, kind="ExternalInput")
    WdTq = nc.dram_tensor("WdTq", [D, 64], BF16, kind="ExternalInput")
    WdTk = nc.dram_tensor("WdTk", [D, 64], BF16, kind="ExternalInput")
    bdq = nc.dram_tensor("bdq", [64, 1], F32, kind="ExternalInput")
    bdk = nc.dram_tensor("bdk", [64, 1], F32, kind="ExternalInput")
    W1pT = nc.dram_tensor("W1pT", [64, P], BF16, kind="ExternalInput")
    W1dT = nc.dram_tensor("W1dT", [64, P], BF16, kind="ExternalInput")
    W1dTf = nc.dram_tensor("W1dTf", [64, P], F32, kind="ExternalInput")
    b1v = nc.dram_tensor("b1v", [P, 1], F32, kind="ExternalInput")
    W2A = nc.dram_tensor("W2A", [P, 65], BF16, kind="ExternalInput")
    onesc = nc.dram_tensor("onesc", [P, 33], BF16, kind="ExternalInput")
    dev_out = nc.dram_tensor("dev_out", [ROWS, 65, L], BF16,
                             kind="ExternalOutput")
    dev_s2 = nc.dram_tensor("dev_s2", [ROWS, L], BF16, kind="ExternalOutput")

    with tile.TileContext(nc) as tc, ExitStack() as ctx:
        const = ctx.enter_context(tc.tile_pool(name="const", bufs=1))
        work = ctx.enter_context(tc.tile_pool(name="work", bufs=4))
        pp1 = ctx.enter_context(tc.tile_pool(name="pp1", bufs=2, space="PSUM"))
        ppA = ctx.enter_context(tc.tile_pool(name="ppA", bufs=2, space="PSUM"))

        # ---- constants into SBUF ----
        xT_sb = const.tile([P, 8, L], BF16)
        nc.sync.dma_start(out=xT_sb, in_=xT[:].rearrange("(c p) l -> p c l", p=P))
        xTr_sb = const.tile([P, 8, ROWS], BF16)
        nc.sync.dma_start(out=xTr_sb, in_=xTr[:].rearrange("(c p) r -> p c r", p=P))
        WdTq_sb = const.tile([P, 8, 64], BF16)
        nc.sync.dma_start(out=WdTq_sb, in_=WdTq[:].rearrange("(c p) m -> p c m", p=P))
        WdTk_sb = const.tile([P, 8, 64], BF16)
        nc.sync.dma_start(out=WdTk_sb, in_=WdTk[:].rearrange("(c p) m -> p c m", p=P))
        bdq_sb = const.tile([64, 1], F32)
        nc.sync.dma_start(out=bdq_sb, in_=bdq[:])
        bdk_sb = const.tile([64, 1], F32)
        nc.sync.dma_start(out=bdk_sb, in_=bdk[:])
        W1pT_sb = const.tile([64, P], BF16)
        nc.sync.dma_start(out=W1pT_sb, in_=W1pT[:])
        W1dTf_sb = const.tile([64, P], F32)
        nc.sync.dma_start(out=W1dTf_sb, in_=W1dTf[:])
        b1v_sb = const.tile([P, 1], F32)
        nc.sync.dma_start(out=b1v_sb, in_=b1v[:])
        W2A_sb = const.tile([P, 65], BF16)
        nc.sync.dma_start(out=W2A_sb, in_=W2A[:])
        ones_sb = const.tile([P, 33], BF16)
        nc.sync.dma_start(out=ones_sb, in_=onesc[:])

        # ---- prep: qq = [q.T; q.T] bf16, kT (local rows), b1c = b1 - W1d@kT
        qq = const.tile([P, L], BF16)
        kT_sb = const.tile([64, ROWS], F32)
        b1c = const.tile([P, ROWS], F32)

        pq = pp1.tile([P, L], F32, tag="p1")
        for c in range(8):
            for h0, h1 in ((0, 512), (512, 768)):
                nc.tensor.matmul(
                    pq[0:64, h0:h1], WdTq_sb[:, c, :], xT_sb[:, c, h0:h1],
                    start=(c == 0), stop=(c == 7),
                )
        nc.scalar.activation(qq[0:64, :], pq[0:64, :], AF.Identity, bias=bdq_sb)
        nc.sync.dma_start(out=qq[64:128, :], in_=qq[0:64, :])

        pk = ppA.tile([P, L], F32, tag="pA")
        for c in range(8):
            nc.tensor.matmul(pk[0:64, 0:ROWS], WdTk_sb[:, c, :], xTr_sb[:, c, :],
                             start=(c == 0), stop=(c == 7))
        nc.scalar.activation(kT_sb, pk[0:64, 0:ROWS], AF.Identity, bias=bdk_sb)

        pc = ppA.tile([P, L], F32, tag="pA")
        nc.tensor.matmul(pc[:, 0:ROWS], W1dTf_sb, kT_sb, start=True, stop=True)
        nc.scalar.activation(b1c, pc[:, 0:ROWS], AF.Identity, bias=b1v_sb,
                             scale=-1.0)

        # persistent W1 stationary tiles (bottom halves static = W1d.T)
        lhsT_t = [const.tile([P, P], BF16, tag=f"lhsT{t}", name=f"lhsT{t}")
                  for t in range(2)]
        for t in range(2):
            nc.sync.dma_start(out=lhsT_t[t][64:128, :], in_=W1dT[:])

        # ---- main loop ----
        for ii in range(ROWS):
            lt = lhsT_t[ii % 2]
            nc.vector.tensor_scalar_mul(lt[0:64, :], W1pT_sb,
                                        kT_sb[:, ii:ii + 1])

            p1 = pp1.tile([P, L], F32, tag="p1", name="p1")
            nc.tensor.matmul(p1[:, 0:512], lt, qq[:, 0:512],
                             start=True, stop=True)
            nc.tensor.matmul(p1[:, 512:768], lt, qq[:, 512:768],
                             start=True, stop=True)

            g = work.tile([P, L], BF16, tag="g", name="g")
            nc.scalar.activation(g, p1, AF.Gelu, bias=b1c[:, ii:ii + 1])
            g2 = work.tile([P, L], BF16, tag="g2", name="g2")
            nc.vector.tensor_mul(g2, g, g)

            pA = ppA.tile([P, L], F32, tag="pA", name="pA")
            # MM-B first: lhsT = [zeros x32 | ones] at base partition 64 puts
            # sum(g^2) on row 96 (rows 64:96 zeroed, then row 64 overwritten
            # by MM-A's sum(g) below).
            nc.tensor.matmul(pA[64:97, 0:512], ones_sb, g2[:, 0:512],
                             start=True, stop=True)
            nc.tensor.matmul(pA[64:97, 512:768], ones_sb, g2[:, 512:768],
                             start=True, stop=True)
            nc.tensor.matmul(pA[0:65, 0:512], W2A_sb, g[:, 0:512],
                             start=True, stop=True)
            nc.tensor.matmul(pA[0:65, 512:768], W2A_sb, g[:, 512:768],
                             start=True, stop=True)

            o_sb = work.tile([97, L], BF16, tag="osb", name="osb")
            nc.vector.tensor_copy(o_sb, pA[0:97, :])
            nc.sync.dma_start(out=dev_out[ii], in_=o_sb[0:65, :])
            nc.sync.dma_start(out=dev_s2[ii], in_=o_sb[96:97, :])


def host_prep(x, W_down, b_down, W1, b1, ln_g, ln_b, W2, b2):
    f32 = np.float32
    bf16 = ml_dtypes.bfloat16
    xTfull = np.ascontiguousarray(x[0].T.astype(f32))  # [D, L]
    W2g = W2.astype(np.float64) * ln_g.astype(np.float64)[None, :]
    W2z = W2g - W2g.mean(axis=1, keepdims=True)  # zero-mean rows absorb LN mu
    W2A = np.concatenate([W2z.T, np.ones((P, 1))], axis=1)  # [128, 65]
    common = {
        "xT": np.ascontiguousarray(xTfull.astype(bf16)),
        "WdTq": np.ascontiguousarray(W_down[:64, :].T.astype(bf16)),
        "WdTk": np.ascontiguousarray(W_down[64:, :].T.astype(bf16)),
        "bdq": np.ascontiguousarray(b_down[:64].astype(f32).reshape(64, 1)),
        "bdk": np.ascontiguousarray(b_down[64:].astype(f32).reshape(64, 1)),
        "W1pT": np.ascontiguousarray(W1[:, :64].T.astype(bf16)),
        "W1dT": np.ascontiguousarray(W1[:, 64:].T.astype(bf16)),
        "W1dTf": np.ascontiguousarray(W1[:, 64:].T.astype(f32)),
        "b1v": np.ascontiguousarray(b1.astype(f32).reshape(P, 1)),
        "W2A": np.ascontiguousarray(W2A.astype(bf16)),
        "onesc": np.ascontiguousarray(
            np.concatenate([np.zeros((P, 32)), np.ones((P, 1))],
                           axis=1).astype(bf16)),
    }
    cvec = (W2.astype(np.float64) @ ln_b.astype(np.float64)
            + b2.astype(np.float64)).astype(f32)
    return common, xTfull, cvec


def kernel(x, W_down, b_down, W1, b1, ln_g, ln_b, W2, b2):
    x = np.asarray(x)
    common, xTfull, cvec = host_prep(
        x, np.asarray(W_down), np.asarray(b_down), np.asarray(W1),
        np.asarray(b1), np.asarray(ln_g), np.asarray(ln_b), np.asarray(W2),
        np.asarray(b2))

    nc = bacc.Bacc("TRN2")
    _build(nc)
    nc.finalize()

    bf16 = ml_dtypes.bfloat16
    in_maps = []
    for core in range(NCORES):
        m = dict(common)
        i0 = core * ROWS
        m["xTr"] = np.ascontiguousarray(xTfull[:, i0:i0 + ROWS].astype(bf16))
        in_maps.append(m)

    trace = os.environ.get("KERNEL_TRACE", "0") == "1"
    res = run_bass_kernel_spmd(nc, in_maps, core_ids=list(range(NCORES)),
                               trace=trace)
    if trace and res.exec_time_ns is not None:
        print(f"HW exec time: {res.exec_time_ns} ns")

    # host tail: LN scale + bias, transpose to [i, j, nb]
    outs = []
    for c in range(NCORES):
        dA = res.results[c]["dev_out"].astype(np.float32)  # [ROWS, 65, L]
        s2 = res.results[c]["dev_s2"].astype(np.float32)   # [ROWS, L]
        po = dA[:, 0:64, :]                     # [ROWS, 64, L]
        mu = dA[:, 64, :] * np.float32(1.0 / 128.0)     # [ROWS, L]
        m2 = s2 * np.float32(1.0 / 128.0)
        var = m2 - mu * mu
        r = 1.0 / np.sqrt(var + np.float32(EPS))        # [ROWS, L]
        out = po.transpose(0, 2, 1) * r[:, :, None] + cvec[None, None, :]
        outs.append(out.astype(np.float32))
    full = np.concatenate(outs, axis=0)  # [768, 768, 64]
    return full[None].astype(np.float32)


# revision 16
# speedup vs baseline: 3.0430x; 1.2673x over previous
"""Trainium2 Bass kernel for nn_PairwisePredictionHead.

Math (reference):
  xd = x @ W_down.T + b_down             # [L, 128]
  q, k = xd[:, :64], xd[:, 64:]
  h[i,j,:] = W1p @ (q_j*k_i) + W1d @ (q_j - k_i) + b1    # [L, L, 128]
  g = gelu_exact(h)
  out = W2 @ LN(g) + b2                   # [L, L, 64]

Sharding: row-shard i across 8 cores (96 rows each). Each core gets the full
q-side (all 768 j) plus its own 96 k-rows; cores are independent (no
collectives), outputs concatenated on host.

Device (per core, per i; all matmuls bf16, 512/256-col splits for PSUM banks):
  - lhsT_i = [[W1pT * k_i[:,None]] ; W1dT]  (top half rebuilt per i on DVE)
  - p1[h, j]   = lhsT_i.T @ [q.T; q.T]            (PE, N=768)
  - g  = Gelu(p1 + (b1 - W1d@k_i))                (ACT, bf16 out)
  - g2 = g*g                                      (DVE, bf16)
  - pA[0:65, j]  = [W2z.T*ln_g | ones].T @ g      (PE; row 64 = sum_h g)
  - pA[96, j]    = ones.T @ g2                    (PE; sum_h g^2)
  - DMA pA[0:65] -> dev_out[i], pA[96:97] -> dev_s2[i]   (fp32, 3KB descs)

Host tail (vectorized numpy): mu = Sg/128, var = Sg2/128 - mu^2,
  r = rsqrt(var+eps), out[i,j,:] = dev_out[i,:,j]*r + (W2@ln_b + b2).
W2z rows are zero-meaned so the matmul absorbs LN's mean subtraction
(w.(g-mu) == (w-mean(w)).g).
"""

import os
from contextlib import ExitStack

import numpy as np
import ml_dtypes

import concourse.bass as bass
import concourse.mybir as mybir
import concourse.tile as tile
from concourse import bacc
from concourse.bass_utils import run_bass_kernel_spmd

F32 = mybir.dt.float32
BF16 = mybir.dt.bfloat16
ALU = mybir.AluOpType
AF = mybir.ActivationFunctionType

B, L, D = 1, 768, 1024
DP, H, NB = 128, 128, 64
NCORES = 8
ROWS = L // NCORES  # 96 pair-grid rows per core
P = 128
EPS = 1e-5


def _build(nc):
    xT = nc.dram_tensor("xT", [D, L], BF16, kind="ExternalInput")
    xTr = nc.dram_tensor("xTr", [D, ROWS], BF16, kind="ExternalInput")
    WdTq = nc.dram_tensor("WdTq", [D, 64], BF16, kind="ExternalInput")
    WdTk = nc.dram_tensor("WdTk", [D, 64], BF16, kind="ExternalInput")
    bdq = nc.dram_tensor("bdq", [64, 1], F32, kind="ExternalInput")
    bdk = nc.dram_tensor("bdk", [64, 1], F32, kind="ExternalInput")
    W1pT = nc.dram_tensor("W1pT", [64, P], BF16, kind="ExternalInput")
    W1dT = nc.dram_tensor("W1dT", [64, P], BF16, kind="ExternalInput")
    W1dTf = nc.dram_tensor("W1dTf", [64, P], F32, kind="ExternalInput")
    b1v = nc.dram_tensor("b1v", [P, 1], F32, kind="ExternalInput")
    W2A = nc.dram_tensor("W2A", [P, 65], BF16, kind="ExternalInput")
    onesc = nc.dram_tensor("onesc", [P, 2], BF16, kind="ExternalInput")
    # partition-major: rows 0:64 = W2z@g, row 64 = sum(g), row 65 = sum(g^2)
    dev_out = nc.dram_tensor("dev_out", [66, ROWS, L], BF16,
                             kind="ExternalOutput")

    with tile.TileContext(nc) as tc, ExitStack() as ctx:
        const = ctx.enter_context(tc.tile_pool(name="const", bufs=1))
        work = ctx.enter_context(tc.tile_pool(name="work", bufs=4))
        pp1 = ctx.enter_context(tc.tile_pool(name="pp1", bufs=2, space="PSUM"))
        ppA = ctx.enter_context(tc.tile_pool(name="ppA", bufs=2, space="PSUM"))

        # ---- constants into SBUF ----
        xT_sb = const.tile([P, 8, L], BF16)
        nc.sync.dma_start(out=xT_sb, in_=xT[:].rearrange("(c p) l -> p c l", p=P))
        xTr_sb = const.tile([P, 8, ROWS], BF16)
        nc.sync.dma_start(out=xTr_sb, in_=xTr[:].rearrange("(c p) r -> p c r", p=P))
        WdTq_sb = const.tile([P, 8, 64], BF16)
        nc.sync.dma_start(out=WdTq_sb, in_=WdTq[:].rearrange("(c p) m -> p c m", p=P))
        WdTk_sb = const.tile([P, 8, 64], BF16)
        nc.sync.dma_start(out=WdTk_sb, in_=WdTk[:].rearrange("(c p) m -> p c m", p=P))
        bdq_sb = const.tile([64, 1], F32)
        nc.sync.dma_start(out=bdq_sb, in_=bdq[:])
        bdk_sb = const.tile([64, 1], F32)
        nc.sync.dma_start(out=bdk_sb, in_=bdk[:])
        W1pT_sb = const.tile([64, P], BF16)
        nc.sync.dma_start(out=W1pT_sb, in_=W1pT[:])
        W1dTf_sb = const.tile([64, P], F32)
        nc.sync.dma_start(out=W1dTf_sb, in_=W1dTf[:])
        b1v_sb = const.tile([P, 1], F32)
        nc.sync.dma_start(out=b1v_sb, in_=b1v[:])
        W2A_sb = const.tile([P, 65], BF16)
        nc.sync.dma_start(out=W2A_sb, in_=W2A[:])
        ones_sb = const.tile([P, 2], BF16)
        nc.sync.dma_start(out=ones_sb, in_=onesc[:])

        # ---- prep: qq = [q.T; q.T] bf16, kT (local rows), b1c = b1 - W1d@kT
        qq = const.tile([P, L], BF16)
        kT_sb = const.tile([64, ROWS], F32)
        b1c = const.tile([P, ROWS], F32)

        pq = pp1.tile([P, L], F32, tag="p1")
        for c in range(8):
            for h0, h1 in ((0, 512), (512, 768)):
                nc.tensor.matmul(
                    pq[0:64, h0:h1], WdTq_sb[:, c, :], xT_sb[:, c, h0:h1],
                    start=(c == 0), stop=(c == 7),
                )
        nc.scalar.activation(qq[0:64, :], pq[0:64, :], AF.Identity, bias=bdq_sb)
        nc.sync.dma_start(out=qq[64:128, :], in_=qq[0:64, :])

        pk = ppA.tile([P, L], F32, tag="pA")
        for c in range(8):
            nc.tensor.matmul(pk[0:64, 0:ROWS], WdTk_sb[:, c, :], xTr_sb[:, c, :],
                             start=(c == 0), stop=(c == 7))
        nc.scalar.activation(kT_sb, pk[0:64, 0:ROWS], AF.Identity, bias=bdk_sb)

        pc = ppA.tile([P, L], F32, tag="pA")
        nc.tensor.matmul(pc[:, 0:ROWS], W1dTf_sb, kT_sb, start=True, stop=True)
        nc.scalar.activation(b1c, pc[:, 0:ROWS], AF.Identity, bias=b1v_sb,
                             scale=-1.0)

        # persistent W1 stationary tiles (bottom halves static = W1d.T)
        lhsT_t = [const.tile([P, P], BF16, tag=f"lhsT{t}", name=f"lhsT{t}")
                  for t in range(2)]
        for t in range(2):
            nc.sync.dma_start(out=lhsT_t[t][64:128, :], in_=W1dT[:])

        # ---- main loop ----
        DB = 4      # i's per output-DMA batch
        CSPL = 384  # psum->SBUF copy column split (DVE | ACT)
        o_sb = None
        for ii in range(ROWS):
            lt = lhsT_t[ii % 2]
            nc.vector.tensor_scalar_mul(lt[0:64, :], W1pT_sb,
                                        kT_sb[:, ii:ii + 1])

            p1 = pp1.tile([P, L], F32, tag="p1", name="p1")
            nc.tensor.matmul(p1[:, 0:512], lt, qq[:, 0:512],
                             start=True, stop=True)
            nc.tensor.matmul(p1[:, 512:768], lt, qq[:, 512:768],
                             start=True, stop=True)

            g = work.tile([P, L], BF16, tag="g", name="g")
            nc.scalar.activation(g, p1, AF.Gelu, bias=b1c[:, ii:ii + 1])
            g2 = work.tile([P, L], BF16, tag="g2", name="g2")
            nc.vector.tensor_mul(g2, g, g)

            pA = ppA.tile([P, L], F32, tag="pA", name="pA")
            # MM-B first: lhsT = [zero-col | ones] at base partition 64 puts
            # sum(g^2) on row 65 (row 64 zeroed here, then overwritten with
            # sum(g) by MM-A below).
            nc.tensor.matmul(pA[64:66, 0:512], ones_sb, g2[:, 0:512],
                             start=True, stop=True)
            nc.tensor.matmul(pA[64:66, 512:768], ones_sb, g2[:, 512:768],
                             start=True, stop=True)
            nc.tensor.matmul(pA[0:65, 0:512], W2A_sb, g[:, 0:512],
                             start=True, stop=True)
            nc.tensor.matmul(pA[0:65, 512:768], W2A_sb, g[:, 512:768],
                             start=True, stop=True)

            bi = ii % DB
            if bi == 0:
                o_sb = work.tile([66, DB, L], BF16, tag="osb", name="osb")
            nc.vector.tensor_copy(o_sb[:, bi, 0:CSPL], pA[0:66, 0:CSPL])
            nc.scalar.copy(o_sb[:, bi, CSPL:L], pA[0:66, CSPL:L])
            if bi == DB - 1:
                i0 = ii - (DB - 1)
                nc.sync.dma_start(
                    out=dev_out[:, i0:i0 + DB, :], in_=o_sb)


def host_prep(x, W_down, b_down, W1, b1, ln_g, ln_b, W2, b2):
    f32 = np.float32
    bf16 = ml_dtypes.bfloat16
    xTfull = np.ascontiguousarray(x[0].T.astype(f32))  # [D, L]
    W2g = W2.astype(np.float64) * ln_g.astype(np.float64)[None, :]
    W2z = W2g - W2g.mean(axis=1, keepdims=True)  # zero-mean rows absorb LN mu
    W2A = np.concatenate([W2z.T, np.ones((P, 1))], axis=1)  # [128, 65]
    common = {
        "xT": np.ascontiguousarray(xTfull.astype(bf16)),
        "WdTq": np.ascontiguousarray(W_down[:64, :].T.astype(bf16)),
        "WdTk": np.ascontiguousarray(W_down[64:, :].T.astype(bf16)),
        "bdq": np.ascontiguousarray(b_down[:64].astype(f32).reshape(64, 1)),
        "bdk": np.ascontiguousarray(b_down[64:].astype(f32).reshape(64, 1)),
        "W1pT": np.ascontiguousarray(W1[:, :64].T.astype(bf16)),
        "W1dT": np.ascontiguousarray(W1[:, 64:].T.astype(bf16)),
        "W1dTf": np.ascontiguousarray(W1[:, 64:].T.astype(f32)),
        "b1v": np.ascontiguousarray(b1.astype(f32).reshape(P, 1)),
        "W2A": np.ascontiguousarray(W2A.astype(bf16)),
        "onesc": np.ascontiguousarray(
            np.concatenate([np.zeros((P, 1)), np.ones((P, 1))],
                           axis=1).astype(bf16)),
    }
    cvec = (W2.astype(np.float64) @ ln_b.astype(np.float64)
            + b2.astype(np.float64)).astype(f32)
    return common, xTfull, cvec


def kernel(x, W_down, b_down, W1, b1, ln_g, ln_b, W2, b2):
    x = np.asarray(x)
    common, xTfull, cvec = host_prep(
        x, np.asarray(W_down), np.asarray(b_down), np.asarray(W1),
        np.asarray(b1), np.asarray(ln_g), np.asarray(ln_b), np.asarray(W2),
        np.asarray(b2))

    nc = bacc.Bacc("TRN2")
    _build(nc)
    nc.finalize()

    bf16 = ml_dtypes.bfloat16
    in_maps = []
    for core in range(NCORES):
        m = dict(common)
        i0 = core * ROWS
        m["xTr"] = np.ascontiguousarray(xTfull[:, i0:i0 + ROWS].astype(bf16))
        in_maps.append(m)

    trace = os.environ.get("KERNEL_TRACE", "0") == "1"
    res = run_bass_kernel_spmd(nc, in_maps, core_ids=list(range(NCORES)),
                               trace=trace)
    if trace and res.exec_time_ns is not None:
        print(f"HW exec time: {res.exec_time_ns} ns")

    # host tail: LN scale + bias, transpose to [i, j, nb]
    outs = []
    for c in range(NCORES):
        dA = res.results[c]["dev_out"].astype(np.float32)  # [66, ROWS, L]
        po = dA[0:64]                                      # [64, ROWS, L]
        mu = dA[64] * np.float32(1.0 / 128.0)              # [ROWS, L]
        m2 = dA[65] * np.float32(1.0 / 128.0)
        var = m2 - mu * mu
        r = 1.0 / np.sqrt(var + np.float32(EPS))           # [ROWS, L]
        out = po.transpose(1, 2, 0) * r[:, :, None] + cvec[None, None, :]
        outs.append(out.astype(np.float32))
    full = np.concatenate(outs, axis=0)  # [768, 768, 64]
    return full[None].astype(np.float32)
